# revision 12
# baseline (speedup 1.0000x reference)
"""Mixtral sparse MoE block on 8 trn2 NeuronCores.

Strategy (expert-parallel, per the sharding hint):
  * Host computes the router (logits -> softmax -> top-k -> dispatch
    indices) in fp32 numpy and gathers each expert's tokens.
  * Core e holds expert e's w1/w3/w2 (packed on host so every DMA is a
    contiguous >=2KB-per-partition line) plus its gathered tokens, and
    computes  (silu(x@w1.T) * (x@w3.T)) @ w2.T  for those tokens.
    Tokens are the moving operand (N<=512 per matmul), weights are the
    stationary operand; hgate stays SBUF-resident per token tile.
  * The router-logits matmul also runs on-device, data-parallel over
    tokens (core i computes logits for tokens [i*T/8, (i+1)*T/8)).
  * Host scatter-adds the routing-weighted expert outputs.

Matmul operands are bf16 (fp32 PSUM accumulation).  Set DT_NAME to
"float32r" for a full-precision variant (slower: more weight DMA).
"""

import math
import os

import numpy as np
import ml_dtypes

import concourse.bass as bass
import concourse.tile as tile
from concourse import mybir
from concourse.bass_utils import run_bass_kernel_spmd

N_CORES = 8
H = 1024          # hidden dim
F = 3584          # ffn dim
NH = H // 128     # 8 h-subtiles
NF = F // 128     # 28 f-subtiles
MM_N = 512        # moving-operand tile (one fp32 PSUM bank)

DT_NAME = os.environ.get("KERNEL_DT", "bfloat16")

# profiling side channel for test.py
LAST_EXEC_NS = None
LAST_RESULTS = None
_LAST_CALL = None  # (nc, in_maps)


def _dt():
    return getattr(mybir.dt, DT_NAME)


def _np_dt():
    return mybir.dt.np(_dt())


def _subtiles(total, step):
    out = []
    o = 0
    while o < total:
        out.append((o, min(step, total - o)))
        o += step
    return out


def _legalize_waits(nc, max_waits=1):
    """This walrus build accepts at most one sync wait per engine
    instruction (setupSyncWait: "Too many sync wait commands"), while Tile
    emits instructions carrying several.  Hoist the extra waits onto
    same-engine nops inserted immediately before the overloaded
    instruction — the engine blocks on each nop in program order, so
    semantics are unchanged."""
    for f in nc.m.functions:
        inserts = {}  # target inst name -> [nop insts]
        created = set()
        for blk in list(f.blocks):
            for inst in list(blk.instructions):
                if inst.name in created:
                    continue
                si = inst.sync_info
                if not (si and si.on_wait and len(si.on_wait) > max_waits):
                    continue
                waits = list(si.on_wait)
                inst.sync_info = mybir.SyncInfo(
                    on_wait=waits[-max_waits:], on_update=list(si.on_update)
                )
                nops = []
                for w in waits[:-max_waits]:
                    n = nc.engines[inst.engine].nop(nofuse=True).ins
                    n.sync_info = mybir.SyncInfo(on_wait=[w], on_update=[])
                    nops.append(n)
                    created.add(n.name)
                inserts[inst.name] = nops
        if not inserts:
            continue
        for blk in list(f.blocks):
            insts = [i for i in blk.instructions if i.name not in created]
            if len(insts) == len(blk.instructions) and not any(
                i.name in inserts for i in insts
            ):
                continue
            out = []
            for i in insts:
                out.extend(inserts.get(i.name, ()))
                out.append(i)
            blk.instructions = out


def _build(C, TDP):
    """Build the SPMD Bass module for per-expert capacity C (tokens,
    multiple of 256) and router slice length TDP."""
    DT = _dt()
    f32 = mybir.dt.float32
    nc = bass.Bass()

    xg = nc.declare_dram_parameter("xg", [NH, 128, C], DT, isOutput=False)
    w1p = nc.declare_dram_parameter("w1p", [NF, 128, H], DT, isOutput=False)
    w3p = nc.declare_dram_parameter("w3p", [NF, 128, H], DT, isOutput=False)
    w2p = nc.declare_dram_parameter("w2p", [NH, 128, F], DT, isOutput=False)
    xdp = nc.declare_dram_parameter("xdp", [NH, 128, TDP], f32, isOutput=False)
    gwp = nc.declare_dram_parameter("gwp", [128, NH * N_CORES], f32, isOutput=False)
    yT = nc.declare_dram_parameter("yT", [NH, 128, C], f32, isOutput=True)
    lg = nc.declare_dram_parameter("lg", [N_CORES, TDP], f32, isOutput=True)

    # token tiles sized so hgate ([128, NF*tokt] DT) stays in SBUF
    max_tokt = 1152 if mybir.dt.size(DT) == 2 else 512
    tok_tiles = _subtiles(C, min(max_tokt, C))

    with tile.TileContext(nc) as tc:
        with (
            tc.tile_pool(name="hgate", bufs=1) as hpool,
            tc.tile_pool(name="xa", bufs=2) as xpool,
            tc.tile_pool(name="wts", bufs=3) as wpool,
            tc.tile_pool(name="w2ts", bufs=2) as w2pool,
            tc.tile_pool(name="tmp", bufs=3) as spool,
            tc.tile_pool(name="outs", bufs=3) as opool,
            tc.tile_pool(name="rtr", bufs=2) as rpool,
            tc.tile_pool(name="psum", bufs=2, space="PSUM") as psum,
            tc.tile_pool(name="psum_r", bufs=1, space="PSUM") as psum_r,
        ):
            # ---- router: logits[t, :] for this core's DP token slice ----
            gwt = rpool.tile([128, NH, N_CORES], f32, tag="gw")
            nc.sync.dma_start(gwt[:], gwp[:].rearrange("p (h e) -> p h e", h=NH))
            for n0, nlen in _subtiles(TDP, MM_N):
                xr = [
                    rpool.tile([128, MM_N], f32, tag="xr", name=f"xr{i}")
                    for i in range(NH)
                ]
                for h in range(NH):
                    nc.sync.dma_start(xr[h][:, :nlen], xdp[h, :, n0 : n0 + nlen])
                pl = psum_r.tile([N_CORES, MM_N], f32, tag="plog")
                for h in range(NH):
                    nc.tensor.matmul(
                        pl[:, :nlen],
                        gwt[:, h, :],
                        xr[h][:, :nlen],
                        start=(h == 0),
                        stop=(h == NH - 1),
                    )
                ls = rpool.tile([N_CORES, MM_N], f32, tag="lsb")
                nc.any.tensor_copy(ls[:, :nlen], pl[:, :nlen])
                nc.sync.dma_start(lg[:, n0 : n0 + nlen], ls[:, :nlen])

            # ---- expert MLP over gathered tokens ----
            for t0, tlen in tok_tiles:
                xt = [
                    xpool.tile([128, tlen], DT, tag=f"xg{i}", name=f"xg{i}")
                    for i in range(NH)
                ]
                for h in range(NH):
                    nc.sync.dma_start(xt[h][:], xg[h, :, t0 : t0 + tlen])
                hgate = hpool.tile([128, NF, tlen], DT, tag="hg")

                # stage 1: hgate[f, t] = silu(w1.x) * (w3.x)
                for ft in range(NF):
                    w1t = wpool.tile([128, H], DT, tag="w1t")
                    nc.sync.dma_start(w1t[:], w1p[ft])
                    w3t = wpool.tile([128, H], DT, tag="w3t")
                    nc.sync.dma_start(w3t[:], w3p[ft])
                    for n0, nlen in _subtiles(tlen, MM_N):
                        p1 = psum.tile([128, MM_N], f32, tag="p1")
                        p3 = psum.tile([128, MM_N], f32, tag="p3")
                        for h in range(NH):
                            nc.tensor.matmul(
                                p1[:, :nlen],
                                w1t[:, h * 128 : (h + 1) * 128],
                                xt[h][:, n0 : n0 + nlen],
                                start=(h == 0),
                                stop=(h == NH - 1),
                            )
                        for h in range(NH):
                            nc.tensor.matmul(
                                p3[:, :nlen],
                                w3t[:, h * 128 : (h + 1) * 128],
                                xt[h][:, n0 : n0 + nlen],
                                start=(h == 0),
                                stop=(h == NH - 1),
                            )
                        tmp = spool.tile([128, MM_N], f32, tag="silu")
                        nc.scalar.activation(
                            tmp[:, :nlen],
                            p1[:, :nlen],
                            mybir.ActivationFunctionType.Silu,
                        )
                        nc.vector.tensor_mul(
                            hgate[:, ft, n0 : n0 + nlen],
                            tmp[:, :nlen],
                            p3[:, :nlen],
                        )

                # stage 2: y[ho, t] = w2 . hgate
                for ho in range(NH):
                    w2t = w2pool.tile([128, F], DT, tag="w2t")
                    nc.sync.dma_start(w2t[:], w2p[ho])
                    for n0, nlen in _subtiles(tlen, MM_N):
                        py = psum.tile([128, MM_N], f32, tag="py")
                        for ft in range(NF):
                            nc.tensor.matmul(
                                py[:, :nlen],
                                w2t[:, ft * 128 : (ft + 1) * 128],
                                hgate[:, ft, n0 : n0 + nlen],
                                start=(ft == 0),
                                stop=(ft == NF - 1),
                            )
                        yt = opool.tile([128, MM_N], f32, tag="yt")
                        nc.any.tensor_copy(yt[:, :nlen], py[:, :nlen])
                        nc.sync.dma_start(
                            yT[ho, :, t0 + n0 : t0 + n0 + nlen], yt[:, :nlen]
                        )

    _legalize_waits(nc)
    return nc


_NC_CACHE = {}


def kernel(hidden_states, gate_w, w1, w2, w3, top_k):
    global LAST_EXEC_NS, LAST_RESULTS
    x = np.ascontiguousarray(np.asarray(hidden_states, dtype=np.float32))
    gate_w = np.asarray(gate_w, dtype=np.float32)
    w1 = np.asarray(w1, dtype=np.float32)
    w2 = np.asarray(w2, dtype=np.float32)
    w3 = np.asarray(w3, dtype=np.float32)
    k = int(top_k)

    b, s, h = x.shape
    T = b * s
    E = gate_w.shape[0]
    assert E == N_CORES and h == H
    xf = x.reshape(T, H)

    # ---- host router (fp32, mirrors the reference op-for-op) ----
    logits = xf @ gate_w.T                                   # [T, E]
    m = logits.max(axis=-1, keepdims=True)
    ex = np.exp(logits - m)
    probs = ex / ex.sum(axis=-1, keepdims=True)
    order = np.argsort(-probs, axis=1, kind="stable")[:, :k]  # [T, k]
    rw = np.take_along_axis(probs, order, axis=1)
    rw = rw / rw.sum(axis=-1, keepdims=True)                  # [T, k]

    flat_tok = np.repeat(np.arange(T), k)
    flat_e = order.ravel()
    flat_w = rw.ravel()
    idx_e, w_e = [], []
    for e in range(E):
        msk = flat_e == e
        idx_e.append(flat_tok[msk])
        w_e.append(flat_w[msk])
    counts = np.array([len(i) for i in idx_e])
    C = max(256, int(math.ceil(counts.max() / 256.0)) * 256)
    TDP = T // N_CORES

    np_dt = _np_dt()
    key = (C, TDP, DT_NAME)
    if key not in _NC_CACHE:
        _NC_CACHE[key] = _build(C, TDP)
    nc = _NC_CACHE[key]

    # ---- pack per-core inputs ----
    gwp = np.ascontiguousarray(gate_w.T.reshape(128 * NH, E)
                               .reshape(NH, 128, E)
                               .transpose(1, 0, 2)
                               .reshape(128, NH * E))
    # gwp[p, h*E + e] = gate_w[e, h*128 + p]
    in_maps = []
    for e in range(E):
        n_e = counts[e]
        xe = np.zeros((C, H), dtype=np.float32)
        xe[:n_e] = xf[idx_e[e]]
        xg = np.ascontiguousarray(xe.T.reshape(NH, 128, C)).astype(np_dt)
        w1T = w1[e].T  # [H, F]
        w1pk = np.ascontiguousarray(
            w1T.reshape(NH, 128, NF, 128).transpose(2, 1, 0, 3).reshape(NF, 128, H)
        ).astype(np_dt)
        w3T = w3[e].T
        w3pk = np.ascontiguousarray(
            w3T.reshape(NH, 128, NF, 128).transpose(2, 1, 0, 3).reshape(NF, 128, H)
        ).astype(np_dt)
        w2T = w2[e].T  # [F, H]
        w2pk = np.ascontiguousarray(
            w2T.reshape(NF, 128, NH, 128).transpose(2, 1, 0, 3).reshape(NH, 128, F)
        ).astype(np_dt)
        xdp = np.ascontiguousarray(
            xf[e * TDP : (e + 1) * TDP].T.reshape(NH, 128, TDP)
        ).astype(np.float32)
        in_maps.append(
            {"xg": xg, "w1p": w1pk, "w3p": w3pk, "w2p": w2pk, "xdp": xdp, "gwp": gwp}
        )

    trace = bool(int(os.environ.get("KERNEL_PROFILE", "0")))
    res = run_bass_kernel_spmd(nc, in_maps, list(range(N_CORES)), trace=trace)
    LAST_EXEC_NS = res.exec_time_ns
    LAST_RESULTS = res
    global _LAST_CALL
    _LAST_CALL = (nc, in_maps)

    # ---- host combine ----
    out = np.zeros((T, H), dtype=np.float32)
    logits_out = np.empty((T, E), dtype=np.float32)
    for e in range(E):
        n_e = counts[e]
        yTe = res.results[e]["yT"].reshape(H, C)       # [H, C]
        ye = yTe.T[:n_e]                               # [n_e, H]
        out[idx_e[e]] += w_e[e][:, None] * ye
        logits_out[e * TDP : (e + 1) * TDP] = res.results[e]["lg"].T
    return out.reshape(b, s, h), logits_out


# ---------------------------------------------------------------------------
# timing utilities (test.py only)
# ---------------------------------------------------------------------------

def _timed_executable(nc, in_maps):
    """jit the prebuilt Bass module over 8 cores WITHOUT buffer donation so
    repeated calls can reuse device-resident inputs; returns (fn, dev_args)."""
    import jax
    from jax.sharding import Mesh, PartitionSpec, NamedSharding
    from jax.experimental.shard_map import shard_map
    from concourse import bass2jax, mybir as _mybir

    bass2jax.install_neuronx_cc_hook()
    n_cores = len(in_maps)
    partition_name = nc.partition_id_tensor.name if nc.partition_id_tensor else None
    in_names, out_names, out_avals, zero_outs = [], [], [], []
    for alloc in nc.m.functions[0].allocations:
        if not isinstance(alloc, _mybir.MemoryLocationSet):
            continue
        name = alloc.memorylocations[0].name
        if alloc.kind == "ExternalInput":
            if name != partition_name:
                in_names.append(name)
        elif alloc.kind == "ExternalOutput":
            out_names.append(name)
            shape = tuple(alloc.tensor_shape)
            dtype = _mybir.dt.np(alloc.dtype)
            out_avals.append(jax.core.ShapedArray(shape, dtype))
            zero_outs.append(np.zeros(shape, dtype))
    n_params = len(in_names)
    all_names = in_names + out_names
    if partition_name is not None:
        all_names = all_names + [partition_name]

    def _body(*args):
        operands = list(args)
        if partition_name is not None:
            operands.append(bass2jax.partition_id_tensor())
        outs = bass2jax._bass_exec_p.bind(
            *operands,
            out_avals=tuple(out_avals),
            in_names=tuple(all_names),
            out_names=tuple(out_names),
            lowering_input_output_aliases=(),
            sim_require_finite=True,
            sim_require_nnan=True,
            nc=nc,
        )
        return tuple(outs)

    devices = jax.devices()[:n_cores]
    mesh = Mesh(np.asarray(devices), ("core",))
    spec = NamedSharding(mesh, PartitionSpec("core"))
    fn = jax.jit(
        shard_map(
            _body,
            mesh=mesh,
            in_specs=(PartitionSpec("core"),) * (n_params + len(out_names)),
            out_specs=(PartitionSpec("core"),) * len(out_names),
            check_rep=False,
        ),
        keep_unused=True,
    )
    concat_in = [
        np.concatenate([np.asarray(in_maps[c][nm]) for c in range(n_cores)], axis=0)
        for nm in in_names
    ] + [np.concatenate([z] * n_cores, axis=0) for z in zero_outs]
    dev_args = [jax.device_put(a, spec) for a in concat_in]
    jax.block_until_ready(dev_args)
    return fn, dev_args


def time_exec(iters=10):
    """Median wall-time (ns) of a device execution of the last-built kernel,
    after a warmup call; inputs stay device-resident."""
    import time as _time
    import jax

    assert _LAST_CALL is not None, "call kernel() first"
    nc, in_maps = _LAST_CALL
    fn, dev_args = _timed_executable(nc, in_maps)
    jax.block_until_ready(fn(*dev_args))  # compile + warm
    times = []
    for _ in range(iters):
        t0 = _time.perf_counter_ns()
        jax.block_until_ready(fn(*dev_args))
        times.append(_time.perf_counter_ns() - t0)
    times.sort()
    return times[len(times) // 2], times


def time_overhead(iters=10):
    """Launch overhead estimate: wall time of a near-empty SPMD kernel."""
    import time as _time
    import jax
    import concourse.bass as _bass

    nc = _bass.Bass()
    x = nc.declare_dram_parameter("x", [128, 128], mybir.dt.float32, isOutput=False)
    y = nc.declare_dram_parameter("y", [128, 128], mybir.dt.float32, isOutput=True)
    with (
        nc.sbuf_tensor([128, 128], mybir.dt.float32) as t,
        nc.semaphore() as sem,
        nc.Block() as block,
    ):
        @block.gpsimd
        def _(gpsimd):
            gpsimd.dma_start(t[:], x[:]).then_inc(sem, 16)
            gpsimd.wait_ge(sem, 16)
            gpsimd.dma_start(y[:], t[:]).then_inc(sem, 16)
            gpsimd.wait_ge(sem, 32)

    arr = np.zeros((128, 128), np.float32)
    fn, dev_args = _timed_executable(nc, [{"x": arr} for _ in range(N_CORES)])
    jax.block_until_ready(fn(*dev_args))
    times = []
    for _ in range(iters):
        t0 = _time.perf_counter_ns()
        jax.block_until_ready(fn(*dev_args))
        times.append(_time.perf_counter_ns() - t0)
    times.sort()
    return times[len(times) // 2], times


# revision 21
# speedup vs baseline: 1.3547x; 1.3547x over previous
"""Mixtral sparse MoE block on 8 trn2 NeuronCores.

Strategy (expert-parallel, per the sharding hint):
  * Host computes the router (logits -> softmax -> top-k -> dispatch
    indices) in fp32 numpy and gathers each expert's tokens.
  * Core e holds expert e's w1/w3/w2 (packed on host so every DMA is a
    contiguous >=2KB-per-partition line) plus its gathered tokens, and
    computes  (silu(x@w1.T) * (x@w3.T)) @ w2.T  for those tokens.
    Tokens are the moving operand (N<=512 per matmul), weights are the
    stationary operand; hgate stays SBUF-resident per token tile.
  * The router-logits matmul also runs on-device, data-parallel over
    tokens (core i computes logits for tokens [i*T/8, (i+1)*T/8)).
  * Host scatter-adds the routing-weighted expert outputs.

Matmul operands are bf16 (fp32 PSUM accumulation).  Set DT_NAME to
"float32r" for a full-precision variant (slower: more weight DMA).
"""

import math
import os

import numpy as np
import ml_dtypes

import concourse.bass as bass
import concourse.tile as tile
from concourse import mybir
from concourse.bass_utils import run_bass_kernel_spmd

N_CORES = 8
H = 1024          # hidden dim
F = 3584          # ffn dim
NH = H // 128     # 8 h-subtiles
NF = F // 128     # 28 f-subtiles
MM_N = 512        # moving-operand tile (one fp32 PSUM bank)

DT_NAME = os.environ.get("KERNEL_DT", "bfloat16")

# profiling side channel for test.py
LAST_EXEC_NS = None
LAST_RESULTS = None
_LAST_CALL = None  # (nc, in_maps)


def _dt():
    return getattr(mybir.dt, DT_NAME)


def _np_dt():
    return mybir.dt.np(_dt())


def _subtiles(total, step):
    out = []
    o = 0
    while o < total:
        out.append((o, min(step, total - o)))
        o += step
    return out


def _legalize_waits(nc, max_waits=1):
    """This walrus build accepts at most one sync wait per engine
    instruction (setupSyncWait: "Too many sync wait commands"), while Tile
    emits instructions carrying several.  Hoist the extra waits onto
    same-engine nops inserted immediately before the overloaded
    instruction — the engine blocks on each nop in program order, so
    semantics are unchanged."""
    for f in nc.m.functions:
        inserts = {}  # target inst name -> [nop insts]
        created = set()
        for blk in list(f.blocks):
            for inst in list(blk.instructions):
                if inst.name in created:
                    continue
                si = inst.sync_info
                if not (si and si.on_wait and len(si.on_wait) > max_waits):
                    continue
                waits = list(si.on_wait)
                inst.sync_info = mybir.SyncInfo(
                    on_wait=waits[-max_waits:], on_update=list(si.on_update)
                )
                nops = []
                for w in waits[:-max_waits]:
                    n = nc.engines[inst.engine].nop(nofuse=True).ins
                    n.sync_info = mybir.SyncInfo(on_wait=[w], on_update=[])
                    nops.append(n)
                    created.add(n.name)
                inserts[inst.name] = nops
        if not inserts:
            continue
        for blk in list(f.blocks):
            insts = [i for i in blk.instructions if i.name not in created]
            if len(insts) == len(blk.instructions) and not any(
                i.name in inserts for i in insts
            ):
                continue
            out = []
            for i in insts:
                out.extend(inserts.get(i.name, ()))
                out.append(i)
            blk.instructions = out


def _build(C, TDP, reps=1):
    """Build the SPMD Bass module for per-expert capacity C (tokens,
    multiple of 256) and router slice length TDP.  reps>1 wraps the body
    in a hardware loop (timing builds only)."""
    import contextlib

    DT = _dt()
    f32 = mybir.dt.float32
    nc = bass.Bass()

    xg = nc.declare_dram_parameter("xg", [NH, 128, C], DT, isOutput=False)
    w1p = nc.declare_dram_parameter("w1p", [NF, 128, H], DT, isOutput=False)
    w3p = nc.declare_dram_parameter("w3p", [NF, 128, H], DT, isOutput=False)
    w2p = nc.declare_dram_parameter("w2p", [NH, 128, F], DT, isOutput=False)
    f32r = mybir.dt.float32r  # fp32 bits, 1 cycle/row on PE at N>=256
    xdp = nc.declare_dram_parameter("xdp", [NH, 128, TDP], f32r, isOutput=False)
    gwp = nc.declare_dram_parameter("gwp", [128, NH * N_CORES], f32r, isOutput=False)
    yT = nc.declare_dram_parameter("yT", [NH, 128, C], f32, isOutput=True)
    lg = nc.declare_dram_parameter("lg", [N_CORES, TDP], f32, isOutput=True)

    # token tiles sized so hgate ([128, NF*tokt] DT) stays in SBUF
    max_tokt = 1152 if mybir.dt.size(DT) == 2 else 512
    tok_tiles = _subtiles(C, min(max_tokt, C))

    with tile.TileContext(nc) as tc:
        with (
            tc.tile_pool(name="hgate", bufs=1) as hpool,
            tc.tile_pool(name="xa", bufs=2) as xpool,
            tc.tile_pool(name="wts", bufs=3) as wpool,
            tc.tile_pool(name="w2ts", bufs=2) as w2pool,
            tc.tile_pool(name="tmp", bufs=3) as spool,
            tc.tile_pool(name="outs", bufs=3) as opool,
            tc.tile_pool(name="rtr", bufs=2) as rpool,
            tc.tile_pool(name="psum", bufs=2, space="PSUM") as psum,
            tc.tile_pool(name="psum_r", bufs=1, space="PSUM") as psum_r,
            tc.For_i(0, reps) if reps > 1 else contextlib.nullcontext(),
        ):
            # ---- router: logits[t, :] for this core's DP token slice ----
            gwt = rpool.tile([128, NH, N_CORES], f32r, tag="gw")
            nc.sync.dma_start(gwt[:], gwp[:].rearrange("p (h e) -> p h e", h=NH))
            for n0, nlen in _subtiles(TDP, MM_N):
                xr = [
                    rpool.tile([128, MM_N], f32r, tag="xr", name=f"xr{i}")
                    for i in range(NH)
                ]
                for h in range(NH):
                    nc.sync.dma_start(xr[h][:, :nlen], xdp[h, :, n0 : n0 + nlen])
                pl = psum_r.tile([N_CORES, MM_N], f32, tag="plog")
                for h in range(NH):
                    nc.tensor.matmul(
                        pl[:, :nlen],
                        gwt[:, h, :],
                        xr[h][:, :nlen],
                        start=(h == 0),
                        stop=(h == NH - 1),
                    )
                ls = rpool.tile([N_CORES, MM_N], f32, tag="lsb")
                nc.any.tensor_copy(ls[:, :nlen], pl[:, :nlen])
                nc.sync.dma_start(lg[:, n0 : n0 + nlen], ls[:, :nlen])

            # ---- expert MLP over gathered tokens ----
            for t0, tlen in tok_tiles:
                xt = [
                    xpool.tile([128, tlen], DT, tag=f"xg{i}", name=f"xg{i}")
                    for i in range(NH)
                ]
                for h in range(NH):
                    nc.sync.dma_start(xt[h][:], xg[h, :, t0 : t0 + tlen])
                hgate = hpool.tile([128, NF, tlen], DT, tag="hg")

                # stage 1: hgate[f, t] = silu(w1.x) * (w3.x)
                for ft in range(NF):
                    w1t = wpool.tile([128, H], DT, tag="w1t")
                    nc.sync.dma_start(w1t[:], w1p[ft])
                    w3t = wpool.tile([128, H], DT, tag="w3t")
                    nc.sync.dma_start(w3t[:], w3p[ft])
                    for n0, nlen in _subtiles(tlen, MM_N):
                        p1 = psum.tile([128, MM_N], f32, tag="p1")
                        p3 = psum.tile([128, MM_N], f32, tag="p3")
                        for h in range(NH):
                            nc.tensor.matmul(
                                p1[:, :nlen],
                                w1t[:, h * 128 : (h + 1) * 128],
                                xt[h][:, n0 : n0 + nlen],
                                start=(h == 0),
                                stop=(h == NH - 1),
                            )
                        for h in range(NH):
                            nc.tensor.matmul(
                                p3[:, :nlen],
                                w3t[:, h * 128 : (h + 1) * 128],
                                xt[h][:, n0 : n0 + nlen],
                                start=(h == 0),
                                stop=(h == NH - 1),
                            )
                        tmp = spool.tile([128, MM_N], f32, tag="silu")
                        nc.scalar.activation(
                            tmp[:, :nlen],
                            p1[:, :nlen],
                            mybir.ActivationFunctionType.Silu,
                        )
                        nc.vector.tensor_mul(
                            hgate[:, ft, n0 : n0 + nlen],
                            tmp[:, :nlen],
                            p3[:, :nlen],
                        )

                # stage 2: y[ho, t] = w2 . hgate
                for ho in range(NH):
                    w2t = w2pool.tile([128, F], DT, tag="w2t")
                    nc.sync.dma_start(w2t[:], w2p[ho])
                    for n0, nlen in _subtiles(tlen, MM_N):
                        py = psum.tile([128, MM_N], f32, tag="py")
                        for ft in range(NF):
                            nc.tensor.matmul(
                                py[:, :nlen],
                                w2t[:, ft * 128 : (ft + 1) * 128],
                                hgate[:, ft, n0 : n0 + nlen],
                                start=(ft == 0),
                                stop=(ft == NF - 1),
                            )
                        yt = opool.tile([128, MM_N], f32, tag="yt")
                        nc.any.tensor_copy(yt[:, :nlen], py[:, :nlen])
                        nc.sync.dma_start(
                            yT[ho, :, t0 + n0 : t0 + n0 + nlen], yt[:, :nlen]
                        )

    _legalize_waits(nc)
    return nc


_NC_CACHE = {}


def kernel(hidden_states, gate_w, w1, w2, w3, top_k):
    global LAST_EXEC_NS, LAST_RESULTS
    x = np.ascontiguousarray(np.asarray(hidden_states, dtype=np.float32))
    gate_w = np.asarray(gate_w, dtype=np.float32)
    w1 = np.asarray(w1, dtype=np.float32)
    w2 = np.asarray(w2, dtype=np.float32)
    w3 = np.asarray(w3, dtype=np.float32)
    k = int(top_k)

    b, s, h = x.shape
    T = b * s
    E = gate_w.shape[0]
    assert E == N_CORES and h == H
    xf = x.reshape(T, H)

    # ---- host router (fp32, mirrors the reference op-for-op) ----
    logits = xf @ gate_w.T                                   # [T, E]
    m = logits.max(axis=-1, keepdims=True)
    ex = np.exp(logits - m)
    probs = ex / ex.sum(axis=-1, keepdims=True)
    order = np.argsort(-probs, axis=1, kind="stable")[:, :k]  # [T, k]
    rw = np.take_along_axis(probs, order, axis=1)
    rw = rw / rw.sum(axis=-1, keepdims=True)                  # [T, k]

    flat_tok = np.repeat(np.arange(T), k)
    flat_e = order.ravel()
    flat_w = rw.ravel()
    idx_e, w_e = [], []
    for e in range(E):
        msk = flat_e == e
        idx_e.append(flat_tok[msk])
        w_e.append(flat_w[msk])
    counts = np.array([len(i) for i in idx_e])
    C = max(256, int(math.ceil(counts.max() / 128.0)) * 128)
    TDP = T // N_CORES

    np_dt = _np_dt()
    key = (C, TDP, DT_NAME)
    if key not in _NC_CACHE:
        _NC_CACHE[key] = _build(C, TDP)
    nc = _NC_CACHE[key]

    # ---- pack per-core inputs ----
    gwp = np.ascontiguousarray(gate_w.T.reshape(128 * NH, E)
                               .reshape(NH, 128, E)
                               .transpose(1, 0, 2)
                               .reshape(128, NH * E))
    # gwp[p, h*E + e] = gate_w[e, h*128 + p]
    in_maps = []
    for e in range(E):
        n_e = counts[e]
        xe = np.zeros((C, H), dtype=np.float32)
        xe[:n_e] = xf[idx_e[e]]
        xg = np.ascontiguousarray(xe.T.reshape(NH, 128, C)).astype(np_dt)
        w1T = w1[e].T  # [H, F]
        w1pk = np.ascontiguousarray(
            w1T.reshape(NH, 128, NF, 128).transpose(2, 1, 0, 3).reshape(NF, 128, H)
        ).astype(np_dt)
        w3T = w3[e].T
        w3pk = np.ascontiguousarray(
            w3T.reshape(NH, 128, NF, 128).transpose(2, 1, 0, 3).reshape(NF, 128, H)
        ).astype(np_dt)
        w2T = w2[e].T  # [F, H]
        w2pk = np.ascontiguousarray(
            w2T.reshape(NF, 128, NH, 128).transpose(2, 1, 0, 3).reshape(NH, 128, F)
        ).astype(np_dt)
        xdp = np.ascontiguousarray(
            xf[e * TDP : (e + 1) * TDP].T.reshape(NH, 128, TDP)
        ).astype(np.float32)
        in_maps.append(
            {"xg": xg, "w1p": w1pk, "w3p": w3pk, "w2p": w2pk, "xdp": xdp, "gwp": gwp}
        )

    trace = bool(int(os.environ.get("KERNEL_PROFILE", "0")))
    res = run_bass_kernel_spmd(nc, in_maps, list(range(N_CORES)), trace=trace)
    LAST_EXEC_NS = res.exec_time_ns
    LAST_RESULTS = res
    global _LAST_CALL
    _LAST_CALL = (nc, in_maps)

    # ---- host combine ----
    out = np.zeros((T, H), dtype=np.float32)
    logits_out = np.empty((T, E), dtype=np.float32)
    for e in range(E):
        n_e = counts[e]
        yTe = res.results[e]["yT"].reshape(H, C)       # [H, C]
        ye = yTe.T[:n_e]                               # [n_e, H]
        out[idx_e[e]] += w_e[e][:, None] * ye
        logits_out[e * TDP : (e + 1) * TDP] = res.results[e]["lg"].T
    return out.reshape(b, s, h), logits_out


# ---------------------------------------------------------------------------
# timing utilities (test.py only)
# ---------------------------------------------------------------------------

def _timed_executable(nc, in_maps):
    """jit the prebuilt Bass module over 8 cores WITHOUT buffer donation so
    repeated calls can reuse device-resident inputs; returns (fn, dev_args)."""
    import jax
    from jax.sharding import Mesh, PartitionSpec, NamedSharding
    from jax.experimental.shard_map import shard_map
    from concourse import bass2jax, mybir as _mybir

    bass2jax.install_neuronx_cc_hook()
    n_cores = len(in_maps)
    partition_name = nc.partition_id_tensor.name if nc.partition_id_tensor else None
    in_names, out_names, out_avals, zero_outs = [], [], [], []
    for alloc in nc.m.functions[0].allocations:
        if not isinstance(alloc, _mybir.MemoryLocationSet):
            continue
        name = alloc.memorylocations[0].name
        if alloc.kind == "ExternalInput":
            if name != partition_name:
                in_names.append(name)
        elif alloc.kind == "ExternalOutput":
            out_names.append(name)
            shape = tuple(alloc.tensor_shape)
            dtype = _mybir.dt.np(alloc.dtype)
            out_avals.append(jax.core.ShapedArray(shape, dtype))
            zero_outs.append(np.zeros(shape, dtype))
    n_params = len(in_names)
    all_names = in_names + out_names
    if partition_name is not None:
        all_names = all_names + [partition_name]

    def _body(*args):
        operands = list(args)
        if partition_name is not None:
            operands.append(bass2jax.partition_id_tensor())
        outs = bass2jax._bass_exec_p.bind(
            *operands,
            out_avals=tuple(out_avals),
            in_names=tuple(all_names),
            out_names=tuple(out_names),
            lowering_input_output_aliases=(),
            sim_require_finite=True,
            sim_require_nnan=True,
            nc=nc,
        )
        return tuple(outs)

    devices = jax.devices()[:n_cores]
    mesh = Mesh(np.asarray(devices), ("core",))
    spec = NamedSharding(mesh, PartitionSpec("core"))
    fn = jax.jit(
        shard_map(
            _body,
            mesh=mesh,
            in_specs=(PartitionSpec("core"),) * (n_params + len(out_names)),
            out_specs=(PartitionSpec("core"),) * len(out_names),
            check_rep=False,
        ),
        keep_unused=True,
    )
    concat_in = [
        np.concatenate([np.asarray(in_maps[c][nm]) for c in range(n_cores)], axis=0)
        for nm in in_names
    ] + [np.concatenate([z] * n_cores, axis=0) for z in zero_outs]
    dev_args = [jax.device_put(a, spec) for a in concat_in]
    jax.block_until_ready(dev_args)
    return fn, dev_args


def time_exec(iters=10):
    """Median wall-time (ns) of a device execution of the last-built kernel,
    after a warmup call; inputs stay device-resident."""
    import time as _time
    import jax

    assert _LAST_CALL is not None, "call kernel() first"
    nc, in_maps = _LAST_CALL
    fn, dev_args = _timed_executable(nc, in_maps)
    jax.block_until_ready(fn(*dev_args))  # compile + warm
    times = []
    for _ in range(iters):
        t0 = _time.perf_counter_ns()
        jax.block_until_ready(fn(*dev_args))
        times.append(_time.perf_counter_ns() - t0)
    times.sort()
    return times[len(times) // 2], times


def time_interleaved(iters=60):
    """Interleave MoE-kernel and near-empty-kernel dispatches; the min-RTT
    difference estimates on-device exec time (launch floor cancels)."""
    import time as _time
    import jax
    import concourse.bass as _bass

    assert _LAST_CALL is not None, "call kernel() first"
    nc, in_maps = _LAST_CALL
    fn_moe, args_moe = _timed_executable(nc, in_maps)

    nc2 = _bass.Bass()
    x = nc2.declare_dram_parameter("x", [128, 128], mybir.dt.float32, isOutput=False)
    y = nc2.declare_dram_parameter("y", [128, 128], mybir.dt.float32, isOutput=True)
    with (
        nc2.sbuf_tensor([128, 128], mybir.dt.float32) as t,
        nc2.semaphore() as sem,
        nc2.Block() as block,
    ):
        @block.gpsimd
        def _(gpsimd):
            gpsimd.dma_start(t[:], x[:]).then_inc(sem, 16)
            gpsimd.wait_ge(sem, 16)
            gpsimd.dma_start(y[:], t[:]).then_inc(sem, 16)
            gpsimd.wait_ge(sem, 32)
    arr = np.zeros((128, 128), np.float32)
    fn_ovh, args_ovh = _timed_executable(nc2, [{"x": arr} for _ in range(N_CORES)])

    np.asarray(fn_moe(*args_moe)[-1])
    np.asarray(fn_ovh(*args_ovh)[-1])
    moe_times, ovh_times = [], []
    for _ in range(iters):
        t0 = _time.perf_counter_ns()
        np.asarray(fn_moe(*args_moe)[-1])
        moe_times.append(_time.perf_counter_ns() - t0)
        t0 = _time.perf_counter_ns()
        np.asarray(fn_ovh(*args_ovh)[-1])
        ovh_times.append(_time.perf_counter_ns() - t0)
    est = max(min(moe_times) - min(ovh_times), 0)
    return est, moe_times, ovh_times


def _timed_fetch_round(fn, args, iters):
    """Dispatch + fetch the small lg output — axon's block_until_ready acks
    on enqueue, so only an output fetch forces device completion."""
    import time as _time

    o = fn(*args)
    np.asarray(o[-1])  # warm
    ts = []
    for _ in range(iters):
        t0 = _time.perf_counter_ns()
        o = fn(*args)
        np.asarray(o[-1])
        ts.append(_time.perf_counter_ns() - t0)
    ts.sort()
    return ts


def time_exec_reps(K=33, iters=12, rounds=3):
    """Precise per-exec time: build a variant with the whole body in a
    hardware loop of K reps; per-exec = (wall_K - wall_1) / (K - 1).
    Completion is forced by fetching an output (see _timed_fetch_round).
    Round 0 is discarded (warmup drift)."""
    assert _LAST_CALL is not None, "call kernel() first"
    nc1, in_maps = _LAST_CALL
    C = in_maps[0]["xg"].shape[2]
    TDP = in_maps[0]["xdp"].shape[2]
    ncK = _build(C, TDP, reps=K)
    fnK, argsK = _timed_executable(ncK, in_maps)
    fn1, args1 = _timed_executable(nc1, in_maps)
    best = None
    details = []
    for rnd in range(rounds):
        t1 = _timed_fetch_round(fn1, args1, iters)
        tK = _timed_fetch_round(fnK, argsK, iters)
        per_exec = (tK[0] - t1[0]) / (K - 1)
        details.append((per_exec, tK[0], t1[0]))
        if rnd > 0 and (best is None or per_exec < best):
            best = per_exec
    return best, details


def time_overhead(iters=10):
    """Launch overhead estimate: wall time of a near-empty SPMD kernel."""
    import time as _time
    import jax
    import concourse.bass as _bass

    nc = _bass.Bass()
    x = nc.declare_dram_parameter("x", [128, 128], mybir.dt.float32, isOutput=False)
    y = nc.declare_dram_parameter("y", [128, 128], mybir.dt.float32, isOutput=True)
    with (
        nc.sbuf_tensor([128, 128], mybir.dt.float32) as t,
        nc.semaphore() as sem,
        nc.Block() as block,
    ):
        @block.gpsimd
        def _(gpsimd):
            gpsimd.dma_start(t[:], x[:]).then_inc(sem, 16)
            gpsimd.wait_ge(sem, 16)
            gpsimd.dma_start(y[:], t[:]).then_inc(sem, 16)
            gpsimd.wait_ge(sem, 32)

    arr = np.zeros((128, 128), np.float32)
    fn, dev_args = _timed_executable(nc, [{"x": arr} for _ in range(N_CORES)])
    jax.block_until_ready(fn(*dev_args))
    times = []
    for _ in range(iters):
        t0 = _time.perf_counter_ns()
        jax.block_until_ready(fn(*dev_args))
        times.append(_time.perf_counter_ns() - t0)
    times.sort()
    return times[len(times) // 2], times
